# revision 19
# baseline (speedup 1.0000x reference)
"""GNN message-passing encoder on 8 Trainium2 NeuronCores.

Computation:
    h      = l2norm(relu(x @ W + b))                    [N, 128]
    neigh1 = segment_mean(h[src], dst)                  [N, 128]
    neigh2 = segment_mean(neigh1[src], dst)             [N, 128]
    out    = (h, 0.7*neigh1 + 0.3*neigh2)

Distribution: nodes range-sharded across 8 cores; each core runs the MLP on
its shard, the f16 feature table is AllGather'd in two halves (so every
gather index fits int16), and each core aggregates the edges whose dst it
owns.  Aggregation = dma_gather of h[src] rows + binary one-hot matmul
segment-sum into PSUM.  The binary one-hot tiles are precomputed on the host
in fp8 (1.0 = 0x38) and streamed from HBM — identical for both hops — and
the 1/deg mean scaling is applied per-node after accumulation, so no DVE
work is needed per edge tile.  SWDGE descriptor scratch is enlarged and
gather calls are sized to fit the ring, so descriptor generation pipelines
with the drains instead of stalling the Pool engine.
"""

import sys

for _p in ("/opt/trn_rl_repo",):
    if _p not in sys.path:
        sys.path.insert(0, _p)

import numpy as np

# ---------------------------------------------------------------- constants
N_NODES = 50000
N_EDGES = 800000
D_IN = 256
D_OUT = 128
NCORES = 8
LAM = 0.7
P = 128

NC_NODES = N_NODES // NCORES            # 6250 nodes per core
HALF = NC_NODES // 2                    # 3125: local-row split for the 2 AG halves
TAB_ROWS = NCORES * HALF                # 25000 rows per half-table (< 32768, int16-safe)
NB = (NC_NODES + P - 1) // P            # 49 dst blocks of 128 nodes per core
SBK = 4                                 # dst blocks per superblock
NSB = (NB + SBK - 1) // SBK             # 13 superblocks
AG_SPLIT_BLOCK = HALF // P              # block 24 finishes rows [0, HALF)
CALL_TILES = 14                         # max tiles per dma_gather call
FP8_ONE = 0x38                          # 1.0 in float8 e4m3

assert TAB_ROWS < 32768
STAGES = "full"  # debug: "mlp", "mlp+ag", "hop1", "full"


# ---------------------------------------------------------------- host prep
def _build_layout(src, dst):
    """Bucket/tile the edge list.  Returns the (core-uniform) tile layout and
    per-core metadata (gather indices, fp8 one-hot tiles, recip columns)."""
    deg = np.bincount(dst, minlength=N_NODES).astype(np.float32)
    recip = (1.0 / np.maximum(deg, 1.0)).astype(np.float32)

    owner = dst // NC_NODES
    per_core = []
    cnt = np.zeros((NCORES, NB, 2), np.int64)
    for c in range(NCORES):
        sel = np.nonzero(owner == c)[0]
        e_dst = dst[sel] - c * NC_NODES
        e_src = src[sel]
        blk = (e_dst >> 7).astype(np.int64)
        dst_mod = (e_dst & 127).astype(np.int64)
        s_c = e_src // NC_NODES
        s_i = e_src % NC_NODES
        grp = (s_i >= HALF).astype(np.int64)
        tabidx = (s_c * HALF + np.where(grp, s_i - HALF, s_i)).astype(np.int16)
        order = np.lexsort((e_src, blk, grp))   # g-major, then block
        blk = blk[order]
        grp = grp[order]
        dst_mod = dst_mod[order]
        tabidx = tabidx[order]
        np.add.at(cnt[c], (blk, grp), 1)
        per_core.append(dict(blk=blk, grp=grp, dst_mod=dst_mod, tabidx=tabidx))

    # core-uniform tile counts per (block, grp)
    mx = cnt.max(axis=0)                       # [NB, 2]
    TG = (mx + P - 1) // P
    for b in range(NB):
        if TG[b].sum() == 0:
            TG[b, 0] = 1

    # enumerate superblocks / tiles / gather calls
    sbs = []
    tau = 0
    ofs = [0, 0]
    for s in range(NSB):
        blocks = list(range(s * SBK, min((s + 1) * SBK, NB)))
        tiles = {b: [] for b in blocks}        # block -> [(tau, slot)]
        slot = 0
        tile_base = {}                         # (b, g) -> first slot
        TgS = [0, 0]
        for g in (0, 1):
            for b in blocks:
                tile_base[(b, g)] = slot
                for _t in range(int(TG[b, g])):
                    tiles[b].append((tau, slot))
                    tau += 1
                    slot += 1
            TgS[g] = slot - (TgS[0] if g == 1 else 0)
        # gather calls: per group, chunks of <= CALL_TILES tiles
        calls = []                             # (g, slot0, ntiles, col0)
        for g in (0, 1):
            g0 = 0 if g == 0 else TgS[0]
            nt_g = TgS[g]
            t0 = 0
            while t0 < nt_g:
                t1 = min(t0 + CALL_TILES, nt_g)
                calls.append((g, g0 + t0, t1 - t0, ofs[g] + t0 * 8))
                t0 = t1
        sbs.append(
            dict(blocks=blocks, TS=slot, TgS=list(TgS), tiles=tiles,
                 tile_base=dict(tile_base), slot0=tau - slot,
                 ofsA=ofs[0], ofsB=ofs[1], calls=calls)
        )
        ofs[0] += TgS[0] * P // 16
        ofs[1] += TgS[1] * P // 16
    T_total = tau
    SIA, SIB = ofs

    # lookup tables indexed by (block, grp)
    base_slot_lut = np.zeros((NB, 2), np.int64)   # slot within sb sequence
    g0_lut = np.zeros((NB, 2), np.int64)          # first slot of grp sequence
    slot0_lut = np.zeros(NB, np.int64)            # global tile of sb start
    ofs_lut = np.zeros((NB, 2), np.int64)         # idx col offset of sb/grp
    for s in range(NSB):
        sb = sbs[s]
        for b in sb["blocks"]:
            slot0_lut[b] = sb["slot0"]
            ofs_lut[b, 0] = sb["ofsA"]
            ofs_lut[b, 1] = sb["ofsB"]
            for g in (0, 1):
                base_slot_lut[b, g] = sb["tile_base"][(b, g)]
                g0_lut[b, g] = 0 if g == 0 else sb["TgS"][0]

    # per-core metadata arrays (vectorized)
    metas = []
    for c in range(NCORES):
        pc = per_core[c]
        blk, grp, dst_mod, tabidx = pc["blk"], pc["grp"], pc["dst_mod"], pc["tabidx"]
        n_e = len(blk)
        # rank of each edge within its (blk, grp) bucket (edges sorted g, blk)
        rank = np.arange(n_e, dtype=np.int64)
        key = grp * NB + blk
        first = np.zeros(n_e, np.int64)
        if n_e:
            starts = np.nonzero(np.diff(key) != 0)[0] + 1
            first[starts] = starts
            first = np.maximum.accumulate(first)
            rank = rank - first

        tile_in_sb = base_slot_lut[blk, grp] + rank // P
        q_in_sbg = (base_slot_lut[blk, grp] - g0_lut[blk, grp]) * P + rank
        glob_tile = slot0_lut[blk] + tile_in_sb

        # gather index tables, 16-row wrapped
        idx_a = np.zeros((16, SIA), np.int16)
        idx_b = np.zeros((16, SIB), np.int16)
        for g, idx_sl in ((0, idx_a), (1, idx_b)):
            m = grp == g
            q = q_in_sbg[m]
            col = ofs_lut[blk[m], g] + q // 16
            idx_sl[q % 16, col] = tabidx[m]

        # fp8 binary one-hot tiles [128, T*128]
        oh8 = np.zeros((P, T_total * P), np.uint8)
        part = rank % P
        oh8[part, glob_tile * P + dst_mod] = FP8_ONE

        # recip columns [128, NB]
        rc = np.zeros((P, NB), np.float32)
        rcol = recip[c * NC_NODES:(c + 1) * NC_NODES]
        rc.T.flat[:NC_NODES] = rcol
        metas.append(
            dict(idx_a=np.tile(idx_a, (8, 1)), idx_b=np.tile(idx_b, (8, 1)),
                 oh8=oh8, recip=rc, recip03=((1.0 - LAM) * rc).astype(np.float32))
        )

    layout = dict(T=T_total, SIA=SIA, SIB=SIB, sbs=sbs, TG=TG)
    return layout, metas


def _layout_key(layout):
    key = [layout["T"], layout["SIA"], layout["SIB"]]
    for sb in layout["sbs"]:
        key += [sb["TS"], sb["ofsA"], sb["ofsB"], tuple(sb["calls"])]
        for b in sb["blocks"]:
            key.append(tuple(t for t, _ in sb["tiles"][b]))
            key.append(tuple(s for _, s in sb["tiles"][b]))
    return tuple(key)


# ---------------------------------------------------------------- device IR
_PROGRAM_CACHE = {}


def _build_program(layout):
    from contextlib import ExitStack

    import concourse.bacc as bacc
    from concourse import mybir
    from concourse.bass import _add_dep_helper
    from concourse.tile import TileContext

    f32 = mybir.dt.float32
    bf16 = mybir.dt.bfloat16
    f16 = mybir.dt.float16
    i16 = mybir.dt.int16
    fp8 = mybir.dt.float8e4
    Alu = mybir.AluOpType
    Act = mybir.ActivationFunctionType

    T = layout["T"]
    SIA = layout["SIA"]
    SIB = layout["SIB"]
    sbs = layout["sbs"]

    nc = bacc.Bacc("TRN2", target_bir_lowering=False, debug=False,
                   num_devices=NCORES, num_swdge_queues=4)

    # I/O
    xt_d = nc.dram_tensor("xt", [NSB, 2, P, SBK * P], bf16, kind="ExternalInput")
    w_d = nc.dram_tensor("wmat", [2, P, D_OUT], bf16, kind="ExternalInput")
    bias_d = nc.dram_tensor("bias", [1, D_OUT], bf16, kind="ExternalInput")
    ones_d = nc.dram_tensor("ones1", [1, P], bf16, kind="ExternalInput")
    eye_d = nc.dram_tensor("eye16", [P, P], f16, kind="ExternalInput")
    oh8_d = nc.dram_tensor("oh8", [P, T * P], fp8, kind="ExternalInput")
    recip_d = nc.dram_tensor("recip", [P, NB], f32, kind="ExternalInput")
    recip03_d = nc.dram_tensor("recip03", [P, NB], f32, kind="ExternalInput")
    idxa_d = nc.dram_tensor("idx_a", [P, SIA], i16, kind="ExternalInput")
    idxb_d = nc.dram_tensor("idx_b", [P, SIB], i16, kind="ExternalInput")

    wu_d = nc.dram_tensor("wu", [P, D_OUT], f16, kind="ExternalInput")
    wuidx_d = nc.dram_tensor("wuidx", [P, 32], i16, kind="ExternalInput")

    h_out_d = nc.dram_tensor("h_out", [NC_NODES, D_OUT], f32, kind="ExternalOutput")
    mh_out_d = nc.dram_tensor("mh_out", [NC_NODES, D_OUT], f32, kind="ExternalOutput")
    wu_out_d = nc.dram_tensor("wu_out", [4, D_OUT], f16, kind="ExternalOutput")

    # internal DRAM
    hshard_d = nc.dram_tensor("hshard16", [NC_NODES, D_OUT], f16)
    n1shard_d = nc.dram_tensor("n1shard16", [NC_NODES, D_OUT], f16)
    htab_a = nc.dram_tensor("htab_a", [TAB_ROWS, D_OUT], f16, addr_space="Shared")
    htab_b = nc.dram_tensor("htab_b", [TAB_ROWS, D_OUT], f16, addr_space="Shared")
    ntab_a = nc.dram_tensor("ntab_a", [TAB_ROWS, D_OUT], f16, addr_space="Shared")
    ntab_b = nc.dram_tensor("ntab_b", [TAB_ROWS, D_OUT], f16, addr_space="Shared")
    # local (non-Shared) copies: SWDGE gather desc-gen is ~2.5x faster when
    # the source table address is in the local DRAM space
    htabl_a = nc.dram_tensor("htabl_a", [TAB_ROWS, D_OUT], f16)
    htabl_b = nc.dram_tensor("htabl_b", [TAB_ROWS, D_OUT], f16)
    ntabl_a = nc.dram_tensor("ntabl_a", [TAB_ROWS, D_OUT], f16)
    ntabl_b = nc.dram_tensor("ntabl_b", [TAB_ROWS, D_OUT], f16)

    rg = [list(range(NCORES))]

    with TileContext(nc) as tc, ExitStack() as ctx:
        const = ctx.enter_context(tc.tile_pool(name="const", bufs=1))
        meta = ctx.enter_context(tc.tile_pool(name="meta", bufs=1))
        xtp = ctx.enter_context(tc.tile_pool(name="xtp", bufs=2))
        featp = ctx.enter_context(tc.tile_pool(name="featp", bufs=3))
        ohp = ctx.enter_context(tc.tile_pool(name="ohp", bufs=3))
        accp = ctx.enter_context(tc.tile_pool(name="accp", bufs=1))
        work = ctx.enter_context(tc.tile_pool(name="work", bufs=3))
        outp = ctx.enter_context(tc.tile_pool(name="outp", bufs=3))
        psmlp = ctx.enter_context(tc.tile_pool(name="psmlp", bufs=3, space="PSUM"))
        pshop = ctx.enter_context(tc.tile_pool(name="pshop", bufs=4, space="PSUM"))

        # ---- constant / metadata loads
        eye_sb = const.tile([P, P], f16, tag="eye16")
        nc.sync.dma_start(eye_sb[:], eye_d[:, :])
        w_sb = [const.tile([P, D_OUT], bf16, tag=f"w{t}", name=f"w_sb{t}")
                for t in range(2)]
        for t in range(2):
            nc.sync.dma_start(w_sb[t][:], w_d[t])
        ones_sb = const.tile([1, P], bf16, tag="ones")
        nc.sync.dma_start(ones_sb[:], ones_d[:, :])
        bias_sb = const.tile([1, D_OUT], bf16, tag="bias")
        nc.sync.dma_start(bias_sb[:], bias_d[:, :])
        recip_sb = const.tile([P, NB], f32, tag="recip")
        nc.sync.dma_start(recip_sb[:], recip_d[:, :])
        recip03_sb = const.tile([P, NB], f32, tag="recip03")
        nc.sync.dma_start(recip03_sb[:], recip03_d[:, :])
        idxa_sb = meta.tile([P, SIA], i16, tag="idxa")
        nc.sync.dma_start(idxa_sb[:], idxa_d[:, :])
        idxb_sb = meta.tile([P, SIB], i16, tag="idxb")
        nc.sync.dma_start(idxb_sb[:], idxb_d[:, :])

        acc_n1 = accp.tile([P, NB * D_OUT], f16, tag="accn1")

        # warm the SWDGE gather ucode path on all 4 queues while the MLP runs
        wuidx_sb = meta.tile([P, 32], i16, tag="wuidx")
        nc.sync.dma_start(wuidx_sb[:], wuidx_d[:, :])
        for q in range(4):
            wf = work.tile([P, 1, D_OUT], f16, tag="wuf")
            nc.gpsimd.dma_gather(
                wf[:, :, :], wu_d[:, :], wuidx_sb[:, q * 8:q * 8 + 8],
                P, nc.gpsimd.to_reg(P), D_OUT, single_packet=False,
                queue_num=q,
            )
            nc.sync.dma_start(wu_out_d[q:q + 1, :], wf[0:1, 0, :])

        # ---- phase 1: MLP  h = l2norm(relu(x @ W + b))
        ag_insts = {}

        def emit_ag(name, src_ap, dst_ap):
            inst = nc.gpsimd.collective_compute(
                "AllGather", Alu.bypass, replica_groups=rg,
                ins=[src_ap], outs=[dst_ap],
            )
            ag_insts[name] = inst
            return inst

        def emit_copy(name, local_ap, dst_ap, eng=None):
            cp = (eng or nc.scalar).dma_start(local_ap, dst_ap)
            _add_dep_helper(cp.ins, ag_insts[name].ins, True, "copy after AG")
            ag_insts[name] = cp

        for s in range(NSB):
            xts = xtp.tile([P, 2, SBK * P], bf16, tag="xts")
            for t in range(2):
                nc.sync.dma_start(xts[:, t, :], xt_d[s, t])
            for bl in range(SBK):
                B = s * SBK + bl
                if B >= NB:
                    break
                ps = psmlp.tile([P, D_OUT], f32, tag="psmlp")
                for t in range(2):
                    nc.tensor.matmul(
                        ps[:], lhsT=xts[:, t, bl * P:(bl + 1) * P],
                        rhs=w_sb[t][:], start=(t == 0), stop=False,
                    )
                nc.tensor.matmul(ps[:], lhsT=ones_sb[:], rhs=bias_sb[:],
                                 start=False, stop=True)
                hb = work.tile([P, D_OUT], f32, tag="hb")
                nc.scalar.activation(hb[:], ps[:], Act.Relu)
                sq = work.tile([P, D_OUT], f32, tag="sq")
                ns = work.tile([P, 1], f32, tag="ns")
                nc.scalar.activation(sq[:], hb[:], Act.Square, accum_out=ns[:])
                nsc = work.tile([P, 1], f32, tag="nsc")
                nc.vector.tensor_scalar(out=nsc[:], in0=ns[:], scalar1=1e-24,
                                        scalar2=None, op0=Alu.max)
                sqr = work.tile([P, 1], f32, tag="sqr")
                nc.scalar.activation(sqr[:], nsc[:], Act.Sqrt)
                rn = work.tile([P, 1], f32, tag="rn")
                nc.vector.reciprocal(rn[:], sqr[:])
                hO = outp.tile([P, D_OUT], f32, tag="hO")
                nc.scalar.activation(hO[:], hb[:], Act.Copy, scale=rn[:])
                h16 = outp.tile([P, D_OUT], f16, tag="h16")
                nc.vector.tensor_scalar(out=h16[:], in0=hb[:], scalar1=rn[:],
                                        scalar2=None, op0=Alu.mult)
                rows = min(P, NC_NODES - B * P)
                nc.sync.dma_start(h_out_d[B * P:B * P + rows, :], hO[:rows, :])
                nc.sync.dma_start(hshard_d[B * P:B * P + rows, :], h16[:rows, :])
                if B == AG_SPLIT_BLOCK and STAGES != "mlp":
                    emit_ag("h_a", hshard_d[0:HALF, :], htab_a[:, :])
        if STAGES != "mlp":
            emit_ag("h_b", hshard_d[HALF:NC_NODES, :], htab_b[:, :])
            emit_copy("h_a", htabl_a[:, :], htab_a[:, :])
            emit_copy("h_b", htabl_b[:, :], htab_b[:, :])

        # ---- phases 2/3: the two aggregation hops
        qctr = [0]
        _size_regs = {}

        def _size_reg(n):
            if n not in _size_regs:
                _size_regs[n] = nc.gpsimd.to_reg(n)
            return _size_regs[n]

        LAG = 1  # A-gathers run one superblock ahead of B-gathers/matmuls

        def emit_hop(tab_a, tab_b, dep_a, dep_b, flush, add_prev=False,
                     post_flush_hook=None):
            live = {}

            def emit_calls(step, grp):
                sb = sbs[step]
                fb, _oh = live[step]
                for g, slot0, ntiles, col0 in sb["calls"]:
                    if g != grp:
                        continue
                    tab = tab_a if g == 0 else tab_b
                    idx_sb = idxa_sb if g == 0 else idxb_sb
                    dep = dep_a if g == 0 else dep_b
                    n = ntiles * P
                    gi = nc.gpsimd.dma_gather(
                        fb[:, slot0:slot0 + ntiles, :], tab[:, :],
                        idx_sb[:, col0:col0 + ntiles * 8],
                        n, _size_reg(n), D_OUT, single_packet=False,
                        queue_num=qctr[0] % 4,
                    )
                    qctr[0] += 1
                    _add_dep_helper(gi.ins, dep.ins, True, "gather after AG")

            for step in range(NSB + LAG):
                if step < NSB:
                    sb = sbs[step]
                    TS = sb["TS"]
                    fb = featp.tile([P, TS, D_OUT], f16, tag="fb")
                    oh = ohp.tile([P, TS, P], fp8, tag="oh")
                    nc.sync.dma_start(
                        oh[:, :, :],
                        oh8_d[:, sb["slot0"] * P:(sb["slot0"] + TS) * P],
                    )
                    live[step] = (fb, oh)
                    emit_calls(step, 0)
                if step < LAG:
                    continue
                s = step - LAG
                sb = sbs[s]
                emit_calls(s, 1)
                fb, oh = live.pop(s)
                for b in sb["blocks"]:
                    tl = sb["tiles"][b]
                    ps = pshop.tile([P, D_OUT], f32, tag="pshop")
                    for i, (tt, slot) in enumerate(tl):
                        nc.tensor.matmul(
                            ps[:], lhsT=oh[:, slot, :], rhs=fb[:, slot, :],
                            start=(i == 0),
                            stop=(not add_prev and i == len(tl) - 1),
                        )
                    if add_prev:
                        # psum += (7/3) * S1 via matmul with I, so the final
                        # recip03 scaling yields 0.7*neigh1 + 0.3*neigh2
                        nc.tensor.matmul(
                            ps[:], lhsT=eye_sb[:],
                            rhs=acc_n1[:, b * D_OUT:(b + 1) * D_OUT],
                            start=False, stop=True,
                        )
                    flush(b, ps)
                if post_flush_hook is not None:
                    post_flush_hook(s)

        if STAGES in ("mlp", "mlp+ag"):
            nc.compile_hook_skip_hops = True

        def flush1(B, ps):
            # acc = (7/3) * S1 (f16), n1 = recip * S1 -> table shard
            nc.scalar.activation(acc_n1[:, B * D_OUT:(B + 1) * D_OUT], ps[:],
                                 Act.Copy, scale=LAM / (1.0 - LAM))
            n16 = outp.tile([P, D_OUT], f16, tag="n16")
            nc.scalar.activation(n16[:], ps[:], Act.Copy,
                                 scale=recip_sb[:, B:B + 1])
            rows = min(P, NC_NODES - B * P)
            nc.sync.dma_start(n1shard_d[B * P:B * P + rows, :], n16[:rows, :])

        def hop1_hook(s):
            if s == 6 and STAGES == "full":
                emit_ag("n_a", n1shard_d[0:HALF, :], ntab_a[:, :])
                emit_copy("n_a", ntabl_a[:, :], ntab_a[:, :])

        if STAGES not in ("mlp", "mlp+ag"):
            emit_hop(htabl_a, htabl_b, ag_insts["h_a"], ag_insts["h_b"],
                     flush1, post_flush_hook=hop1_hook)
        if STAGES == "full":
            emit_ag("n_b", n1shard_d[HALF:NC_NODES, :], ntab_b[:, :])
            emit_copy("n_b", ntabl_b[:, :], ntab_b[:, :])

        def flush2(B, ps):
            mh = outp.tile([P, D_OUT], f32, tag="mh")
            nc.scalar.activation(mh[:], ps[:], Act.Copy,
                                 scale=recip03_sb[:, B:B + 1])
            rows = min(P, NC_NODES - B * P)
            nc.sync.dma_start(mh_out_d[B * P:B * P + rows, :], mh[:rows, :])

        if STAGES == "full":
            emit_hop(ntabl_a, ntabl_b, ag_insts["n_a"], ag_insts["n_b"],
                     flush2, add_prev=True)

    nc.compile()
    return nc


# ---------------------------------------------------------------- entry
def _build_in_maps(x, W, b, metas):
    import ml_dtypes

    eye16 = np.eye(P).astype(np.float16)
    wmat = np.stack([W[0:P, :], W[P:2 * P, :]]).astype(ml_dtypes.bfloat16)
    bias = b.reshape(1, D_OUT).astype(ml_dtypes.bfloat16)
    ones1 = np.ones((1, P), ml_dtypes.bfloat16)

    in_maps = []
    for c in range(NCORES):
        xs = x[c * NC_NODES:(c + 1) * NC_NODES]
        xs_pad = np.zeros((NSB * SBK * P, D_IN), np.float32)
        xs_pad[:NC_NODES] = xs
        xt = np.zeros((NSB, 2, P, SBK * P), ml_dtypes.bfloat16)
        for s in range(NSB):
            chunk = xs_pad[s * SBK * P:(s + 1) * SBK * P]  # [512, 256]
            ct = np.ascontiguousarray(chunk.T)             # [256, 512]
            xt[s, 0] = ct[0:P]
            xt[s, 1] = ct[P:2 * P]
        m = metas[c]
        in_maps.append(
            dict(
                xt=xt, wmat=wmat, bias=bias, ones1=ones1, eye16=eye16,
                oh8=m["oh8"].view(ml_dtypes.float8_e4m3),
                recip=m["recip"], recip03=m["recip03"],
                idx_a=m["idx_a"], idx_b=m["idx_b"],
                wu=np.zeros((P, D_OUT), np.float16),
                wuidx=np.zeros((P, 32), np.int16),
            )
        )
    return in_maps


def kernel(x, W, b, src, dst):
    x = np.asarray(x, np.float32)
    W = np.asarray(W, np.float32)
    b = np.asarray(b, np.float32)
    src = np.asarray(src, np.int32)
    dst = np.asarray(dst, np.int32)

    layout, metas = _build_layout(src, dst)
    key = _layout_key(layout)
    if key not in _PROGRAM_CACHE:
        _PROGRAM_CACHE[key] = _build_program(layout)
    nc = _PROGRAM_CACHE[key]
    in_maps = _build_in_maps(x, W, b, metas)

    from concourse.bass_utils import run_bass_kernel_spmd

    res = run_bass_kernel_spmd(nc, in_maps, list(range(NCORES)))
    h = np.concatenate([res.results[c]["h_out"] for c in range(NCORES)], axis=0)
    mh = np.concatenate([res.results[c]["mh_out"] for c in range(NCORES)], axis=0)
    return (h, mh)


# revision 20
# speedup vs baseline: 1.0338x; 1.0338x over previous
"""GNN message-passing encoder on 8 Trainium2 NeuronCores.

Computation:
    h      = l2norm(relu(x @ W + b))                    [N, 128]
    neigh1 = segment_mean(h[src], dst)                  [N, 128]
    neigh2 = segment_mean(neigh1[src], dst)             [N, 128]
    out    = (h, 0.7*neigh1 + 0.3*neigh2)

Distribution: nodes range-sharded across 8 cores; each core runs the MLP on
its shard, the f16 feature table is AllGather'd in two halves (so every
gather index fits int16), and each core aggregates the edges whose dst it
owns.  Aggregation = dma_gather of h[src] rows + binary one-hot matmul
segment-sum into PSUM.  The binary one-hot tiles are precomputed on the host
in fp8 (1.0 = 0x38) and streamed from HBM — identical for both hops — and
the 1/deg mean scaling is applied per-node after accumulation, so no DVE
work is needed per edge tile.  SWDGE descriptor scratch is enlarged and
gather calls are sized to fit the ring, so descriptor generation pipelines
with the drains instead of stalling the Pool engine.
"""

import sys

for _p in ("/opt/trn_rl_repo",):
    if _p not in sys.path:
        sys.path.insert(0, _p)

import numpy as np

# ---------------------------------------------------------------- constants
N_NODES = 50000
N_EDGES = 800000
D_IN = 256
D_OUT = 128
NCORES = 8
LAM = 0.7
P = 128

NC_NODES = N_NODES // NCORES            # 6250 nodes per core
HALF = 4096                             # A-half rows per core (max idx 7*4096+4095 = 32767)
HALF_B = NC_NODES - HALF                # 2154 B-half rows per core
TAB_ROWS = NCORES * HALF                # 32768 rows in the A table
TAB_ROWS_B = NCORES * HALF_B            # 17232 rows in the B table
NB = (NC_NODES + P - 1) // P            # 49 dst blocks of 128 nodes per core
SBK = 4                                 # dst blocks per superblock
NSB = (NB + SBK - 1) // SBK             # 13 superblocks
AG_SPLIT_BLOCK = HALF // P - 1          # block 31 finishes rows [0, HALF)
CALL_TILES = 14                         # max tiles per dma_gather call
FP8_ONE = 0x38                          # 1.0 in float8 e4m3

assert (NCORES - 1) * HALF + HALF - 1 < 32768
assert (NCORES - 1) * HALF_B + HALF_B - 1 < 32768
STAGES = "full"  # debug: "mlp", "mlp+ag", "hop1", "full"


# ---------------------------------------------------------------- host prep
def _build_layout(src, dst):
    """Bucket/tile the edge list.  Returns the (core-uniform) tile layout and
    per-core metadata (gather indices, fp8 one-hot tiles, recip columns)."""
    deg = np.bincount(dst, minlength=N_NODES).astype(np.float32)
    recip = (1.0 / np.maximum(deg, 1.0)).astype(np.float32)

    owner = dst // NC_NODES
    per_core = []
    cnt = np.zeros((NCORES, NB, 2), np.int64)
    for c in range(NCORES):
        sel = np.nonzero(owner == c)[0]
        e_dst = dst[sel] - c * NC_NODES
        e_src = src[sel]
        blk = (e_dst >> 7).astype(np.int64)
        dst_mod = (e_dst & 127).astype(np.int64)
        s_c = e_src // NC_NODES
        s_i = e_src % NC_NODES
        grp = (s_i >= HALF).astype(np.int64)
        tabidx = np.where(grp, s_c * HALF_B + (s_i - HALF),
                          s_c * HALF + s_i).astype(np.int16)
        order = np.lexsort((e_src, blk, grp))   # g-major, then block
        blk = blk[order]
        grp = grp[order]
        dst_mod = dst_mod[order]
        tabidx = tabidx[order]
        np.add.at(cnt[c], (blk, grp), 1)
        per_core.append(dict(blk=blk, grp=grp, dst_mod=dst_mod, tabidx=tabidx))

    # core-uniform tile counts per (block, grp)
    mx = cnt.max(axis=0)                       # [NB, 2]
    TG = (mx + P - 1) // P
    for b in range(NB):
        if TG[b].sum() == 0:
            TG[b, 0] = 1

    # enumerate superblocks / tiles / gather calls
    sbs = []
    tau = 0
    ofs = [0, 0]
    for s in range(NSB):
        blocks = list(range(s * SBK, min((s + 1) * SBK, NB)))
        tiles = {b: [] for b in blocks}        # block -> [(tau, slot)]
        slot = 0
        tile_base = {}                         # (b, g) -> first slot
        TgS = [0, 0]
        for g in (0, 1):
            for b in blocks:
                tile_base[(b, g)] = slot
                for _t in range(int(TG[b, g])):
                    tiles[b].append((tau, slot))
                    tau += 1
                    slot += 1
            TgS[g] = slot - (TgS[0] if g == 1 else 0)
        # gather calls: per group, chunks of <= CALL_TILES tiles
        calls = []                             # (g, slot0, ntiles, col0)
        for g in (0, 1):
            g0 = 0 if g == 0 else TgS[0]
            nt_g = TgS[g]
            t0 = 0
            while t0 < nt_g:
                t1 = min(t0 + CALL_TILES, nt_g)
                calls.append((g, g0 + t0, t1 - t0, ofs[g] + t0 * 8))
                t0 = t1
        sbs.append(
            dict(blocks=blocks, TS=slot, TgS=list(TgS), tiles=tiles,
                 tile_base=dict(tile_base), slot0=tau - slot,
                 ofsA=ofs[0], ofsB=ofs[1], calls=calls)
        )
        ofs[0] += TgS[0] * P // 16
        ofs[1] += TgS[1] * P // 16
    T_total = tau
    SIA, SIB = ofs

    # lookup tables indexed by (block, grp)
    base_slot_lut = np.zeros((NB, 2), np.int64)   # slot within sb sequence
    g0_lut = np.zeros((NB, 2), np.int64)          # first slot of grp sequence
    slot0_lut = np.zeros(NB, np.int64)            # global tile of sb start
    ofs_lut = np.zeros((NB, 2), np.int64)         # idx col offset of sb/grp
    for s in range(NSB):
        sb = sbs[s]
        for b in sb["blocks"]:
            slot0_lut[b] = sb["slot0"]
            ofs_lut[b, 0] = sb["ofsA"]
            ofs_lut[b, 1] = sb["ofsB"]
            for g in (0, 1):
                base_slot_lut[b, g] = sb["tile_base"][(b, g)]
                g0_lut[b, g] = 0 if g == 0 else sb["TgS"][0]

    # per-core metadata arrays (vectorized)
    metas = []
    for c in range(NCORES):
        pc = per_core[c]
        blk, grp, dst_mod, tabidx = pc["blk"], pc["grp"], pc["dst_mod"], pc["tabidx"]
        n_e = len(blk)
        # rank of each edge within its (blk, grp) bucket (edges sorted g, blk)
        rank = np.arange(n_e, dtype=np.int64)
        key = grp * NB + blk
        first = np.zeros(n_e, np.int64)
        if n_e:
            starts = np.nonzero(np.diff(key) != 0)[0] + 1
            first[starts] = starts
            first = np.maximum.accumulate(first)
            rank = rank - first

        tile_in_sb = base_slot_lut[blk, grp] + rank // P
        q_in_sbg = (base_slot_lut[blk, grp] - g0_lut[blk, grp]) * P + rank
        glob_tile = slot0_lut[blk] + tile_in_sb

        # gather index tables, 16-row wrapped
        idx_a = np.zeros((16, SIA), np.int16)
        idx_b = np.zeros((16, SIB), np.int16)
        for g, idx_sl in ((0, idx_a), (1, idx_b)):
            m = grp == g
            q = q_in_sbg[m]
            col = ofs_lut[blk[m], g] + q // 16
            idx_sl[q % 16, col] = tabidx[m]

        # fp8 binary one-hot tiles [128, T*128]
        oh8 = np.zeros((P, T_total * P), np.uint8)
        part = rank % P
        oh8[part, glob_tile * P + dst_mod] = FP8_ONE

        # recip columns [128, NB]
        rc = np.zeros((P, NB), np.float32)
        rcol = recip[c * NC_NODES:(c + 1) * NC_NODES]
        rc.T.flat[:NC_NODES] = rcol
        metas.append(
            dict(idx_a=np.tile(idx_a, (8, 1)), idx_b=np.tile(idx_b, (8, 1)),
                 oh8=oh8, recip=rc, recip03=((1.0 - LAM) * rc).astype(np.float32))
        )

    layout = dict(T=T_total, SIA=SIA, SIB=SIB, sbs=sbs, TG=TG)
    return layout, metas


def _layout_key(layout):
    key = [layout["T"], layout["SIA"], layout["SIB"]]
    for sb in layout["sbs"]:
        key += [sb["TS"], sb["ofsA"], sb["ofsB"], tuple(sb["calls"])]
        for b in sb["blocks"]:
            key.append(tuple(t for t, _ in sb["tiles"][b]))
            key.append(tuple(s for _, s in sb["tiles"][b]))
    return tuple(key)


# ---------------------------------------------------------------- device IR
_PROGRAM_CACHE = {}


def _build_program(layout):
    from contextlib import ExitStack

    import concourse.bacc as bacc
    from concourse import mybir
    from concourse.bass import _add_dep_helper
    from concourse.tile import TileContext

    f32 = mybir.dt.float32
    bf16 = mybir.dt.bfloat16
    f16 = mybir.dt.float16
    i16 = mybir.dt.int16
    fp8 = mybir.dt.float8e4
    Alu = mybir.AluOpType
    Act = mybir.ActivationFunctionType

    T = layout["T"]
    SIA = layout["SIA"]
    SIB = layout["SIB"]
    sbs = layout["sbs"]

    nc = bacc.Bacc("TRN2", target_bir_lowering=False, debug=False,
                   num_devices=NCORES, num_swdge_queues=4)

    # I/O
    xt_d = nc.dram_tensor("xt", [NSB, 2, P, SBK * P], bf16, kind="ExternalInput")
    w_d = nc.dram_tensor("wmat", [2, P, D_OUT], bf16, kind="ExternalInput")
    bias_d = nc.dram_tensor("bias", [1, D_OUT], bf16, kind="ExternalInput")
    ones_d = nc.dram_tensor("ones1", [1, P], bf16, kind="ExternalInput")
    eye_d = nc.dram_tensor("eye16", [P, P], f16, kind="ExternalInput")
    oh8_d = nc.dram_tensor("oh8", [P, T * P], fp8, kind="ExternalInput")
    recip_d = nc.dram_tensor("recip", [P, NB], f32, kind="ExternalInput")
    recip03_d = nc.dram_tensor("recip03", [P, NB], f32, kind="ExternalInput")
    idxa_d = nc.dram_tensor("idx_a", [P, SIA], i16, kind="ExternalInput")
    idxb_d = nc.dram_tensor("idx_b", [P, SIB], i16, kind="ExternalInput")

    wu_d = nc.dram_tensor("wu", [P, D_OUT], f16, kind="ExternalInput")
    wuidx_d = nc.dram_tensor("wuidx", [P, 32], i16, kind="ExternalInput")

    h_out_d = nc.dram_tensor("h_out", [NC_NODES, D_OUT], f32, kind="ExternalOutput")
    mh_out_d = nc.dram_tensor("mh_out", [NC_NODES, D_OUT], f32, kind="ExternalOutput")
    wu_out_d = nc.dram_tensor("wu_out", [4, D_OUT], f16, kind="ExternalOutput")

    # internal DRAM
    hshard_d = nc.dram_tensor("hshard16", [NC_NODES, D_OUT], f16)
    n1shard_d = nc.dram_tensor("n1shard16", [NC_NODES, D_OUT], f16)
    htab_a = nc.dram_tensor("htab_a", [TAB_ROWS, D_OUT], f16, addr_space="Shared")
    htab_b = nc.dram_tensor("htab_b", [TAB_ROWS_B, D_OUT], f16, addr_space="Shared")
    ntab_a = nc.dram_tensor("ntab_a", [TAB_ROWS, D_OUT], f16, addr_space="Shared")
    ntab_b = nc.dram_tensor("ntab_b", [TAB_ROWS_B, D_OUT], f16, addr_space="Shared")
    # local (non-Shared) copies: SWDGE gather desc-gen is ~2.5x faster when
    # the source table address is in the local DRAM space
    htabl_a = nc.dram_tensor("htabl_a", [TAB_ROWS, D_OUT], f16)
    htabl_b = nc.dram_tensor("htabl_b", [TAB_ROWS_B, D_OUT], f16)
    ntabl_a = nc.dram_tensor("ntabl_a", [TAB_ROWS, D_OUT], f16)
    ntabl_b = nc.dram_tensor("ntabl_b", [TAB_ROWS_B, D_OUT], f16)

    rg = [list(range(NCORES))]

    with TileContext(nc) as tc, ExitStack() as ctx:
        const = ctx.enter_context(tc.tile_pool(name="const", bufs=1))
        meta = ctx.enter_context(tc.tile_pool(name="meta", bufs=1))
        xtp = ctx.enter_context(tc.tile_pool(name="xtp", bufs=2))
        featp = ctx.enter_context(tc.tile_pool(name="featp", bufs=3))
        ohp = ctx.enter_context(tc.tile_pool(name="ohp", bufs=3))
        accp = ctx.enter_context(tc.tile_pool(name="accp", bufs=1))
        work = ctx.enter_context(tc.tile_pool(name="work", bufs=3))
        outp = ctx.enter_context(tc.tile_pool(name="outp", bufs=3))
        psmlp = ctx.enter_context(tc.tile_pool(name="psmlp", bufs=3, space="PSUM"))
        pshop = ctx.enter_context(tc.tile_pool(name="pshop", bufs=4, space="PSUM"))

        # ---- constant / metadata loads
        eye_sb = const.tile([P, P], f16, tag="eye16")
        nc.sync.dma_start(eye_sb[:], eye_d[:, :])
        w_sb = [const.tile([P, D_OUT], bf16, tag=f"w{t}", name=f"w_sb{t}")
                for t in range(2)]
        for t in range(2):
            nc.sync.dma_start(w_sb[t][:], w_d[t])
        ones_sb = const.tile([1, P], bf16, tag="ones")
        nc.sync.dma_start(ones_sb[:], ones_d[:, :])
        bias_sb = const.tile([1, D_OUT], bf16, tag="bias")
        nc.sync.dma_start(bias_sb[:], bias_d[:, :])
        recip_sb = const.tile([P, NB], f32, tag="recip")
        nc.sync.dma_start(recip_sb[:], recip_d[:, :])
        recip03_sb = const.tile([P, NB], f32, tag="recip03")
        nc.sync.dma_start(recip03_sb[:], recip03_d[:, :])
        idxa_sb = meta.tile([P, SIA], i16, tag="idxa")
        nc.sync.dma_start(idxa_sb[:], idxa_d[:, :])
        idxb_sb = meta.tile([P, SIB], i16, tag="idxb")
        nc.sync.dma_start(idxb_sb[:], idxb_d[:, :])

        acc_n1 = accp.tile([P, NB * D_OUT], f16, tag="accn1")

        # warm the SWDGE gather ucode path on all 4 queues while the MLP runs
        wuidx_sb = meta.tile([P, 32], i16, tag="wuidx")
        nc.sync.dma_start(wuidx_sb[:], wuidx_d[:, :])
        for q in range(4):
            wf = work.tile([P, 1, D_OUT], f16, tag="wuf")
            nc.gpsimd.dma_gather(
                wf[:, :, :], wu_d[:, :], wuidx_sb[:, q * 8:q * 8 + 8],
                P, nc.gpsimd.to_reg(P), D_OUT, single_packet=False,
                queue_num=q,
            )
            nc.sync.dma_start(wu_out_d[q:q + 1, :], wf[0:1, 0, :])

        # ---- phase 1: MLP  h = l2norm(relu(x @ W + b))
        ag_insts = {}

        def emit_ag(name, src_ap, dst_ap):
            inst = nc.gpsimd.collective_compute(
                "AllGather", Alu.bypass, replica_groups=rg,
                ins=[src_ap], outs=[dst_ap],
            )
            ag_insts[name] = inst
            return inst

        def emit_copy(name, local_ap, dst_ap, eng=None):
            cp = (eng or nc.scalar).dma_start(local_ap, dst_ap)
            _add_dep_helper(cp.ins, ag_insts[name].ins, True, "copy after AG")
            ag_insts[name] = cp

        for s in range(NSB):
            xts = xtp.tile([P, 2, SBK * P], bf16, tag="xts")
            for t in range(2):
                nc.sync.dma_start(xts[:, t, :], xt_d[s, t])
            for bl in range(SBK):
                B = s * SBK + bl
                if B >= NB:
                    break
                ps = psmlp.tile([P, D_OUT], f32, tag="psmlp")
                for t in range(2):
                    nc.tensor.matmul(
                        ps[:], lhsT=xts[:, t, bl * P:(bl + 1) * P],
                        rhs=w_sb[t][:], start=(t == 0), stop=False,
                    )
                nc.tensor.matmul(ps[:], lhsT=ones_sb[:], rhs=bias_sb[:],
                                 start=False, stop=True)
                hb = work.tile([P, D_OUT], f32, tag="hb")
                nc.scalar.activation(hb[:], ps[:], Act.Relu)
                sq = work.tile([P, D_OUT], f32, tag="sq")
                ns = work.tile([P, 1], f32, tag="ns")
                nc.scalar.activation(sq[:], hb[:], Act.Square, accum_out=ns[:])
                nsc = work.tile([P, 1], f32, tag="nsc")
                nc.vector.tensor_scalar(out=nsc[:], in0=ns[:], scalar1=1e-24,
                                        scalar2=None, op0=Alu.max)
                sqr = work.tile([P, 1], f32, tag="sqr")
                nc.scalar.activation(sqr[:], nsc[:], Act.Sqrt)
                rn = work.tile([P, 1], f32, tag="rn")
                nc.vector.reciprocal(rn[:], sqr[:])
                hO = outp.tile([P, D_OUT], f32, tag="hO")
                nc.scalar.activation(hO[:], hb[:], Act.Copy, scale=rn[:])
                h16 = outp.tile([P, D_OUT], f16, tag="h16")
                nc.vector.tensor_scalar(out=h16[:], in0=hb[:], scalar1=rn[:],
                                        scalar2=None, op0=Alu.mult)
                rows = min(P, NC_NODES - B * P)
                nc.sync.dma_start(h_out_d[B * P:B * P + rows, :], hO[:rows, :])
                nc.sync.dma_start(hshard_d[B * P:B * P + rows, :], h16[:rows, :])
                if B == AG_SPLIT_BLOCK and STAGES != "mlp":
                    emit_ag("h_a", hshard_d[0:HALF, :], htab_a[:, :])
        if STAGES != "mlp":
            emit_ag("h_b", hshard_d[HALF:NC_NODES, :], htab_b[:, :])
            emit_copy("h_a", htabl_a[:, :], htab_a[:, :])
            emit_copy("h_b", htabl_b[:, :], htab_b[:, :])

        # ---- phases 2/3: the two aggregation hops
        qctr = [0]
        _size_regs = {}

        def _size_reg(n):
            if n not in _size_regs:
                _size_regs[n] = nc.gpsimd.to_reg(n)
            return _size_regs[n]

        def emit_hop(tab_a, tab_b, dep_a, dep_b, flush, add_prev=False,
                     post_flush_hook=None):
            for s in range(NSB):
                sb = sbs[s]
                TS = sb["TS"]
                fb = featp.tile([P, TS, D_OUT], f16, tag="fb")
                oh = ohp.tile([P, TS, P], fp8, tag="oh")
                nc.sync.dma_start(
                    oh[:, :, :],
                    oh8_d[:, sb["slot0"] * P:(sb["slot0"] + TS) * P],
                )
                for g, slot0, ntiles, col0 in sb["calls"]:
                    tab = tab_a if g == 0 else tab_b
                    idx_sb = idxa_sb if g == 0 else idxb_sb
                    dep = dep_a if g == 0 else dep_b
                    n = ntiles * P
                    gi = nc.gpsimd.dma_gather(
                        fb[:, slot0:slot0 + ntiles, :], tab[:, :],
                        idx_sb[:, col0:col0 + ntiles * 8],
                        n, _size_reg(n), D_OUT, single_packet=False,
                        queue_num=qctr[0] % 4,
                    )
                    qctr[0] += 1
                    _add_dep_helper(gi.ins, dep.ins, True, "gather after AG")
                for b in sb["blocks"]:
                    tl = sb["tiles"][b]
                    ps = pshop.tile([P, D_OUT], f32, tag="pshop")
                    for i, (tt, slot) in enumerate(tl):
                        nc.tensor.matmul(
                            ps[:], lhsT=oh[:, slot, :], rhs=fb[:, slot, :],
                            start=(i == 0),
                            stop=(not add_prev and i == len(tl) - 1),
                        )
                    if add_prev:
                        # psum += (7/3) * S1 via matmul with I, so the final
                        # recip03 scaling yields 0.7*neigh1 + 0.3*neigh2
                        nc.tensor.matmul(
                            ps[:], lhsT=eye_sb[:],
                            rhs=acc_n1[:, b * D_OUT:(b + 1) * D_OUT],
                            start=False, stop=True,
                        )
                    flush(b, ps)
                if post_flush_hook is not None:
                    post_flush_hook(s)

        if STAGES in ("mlp", "mlp+ag"):
            nc.compile_hook_skip_hops = True

        def flush1(B, ps):
            # acc = (7/3) * S1 (f16), n1 = recip * S1 -> table shard
            nc.scalar.activation(acc_n1[:, B * D_OUT:(B + 1) * D_OUT], ps[:],
                                 Act.Copy, scale=LAM / (1.0 - LAM))
            n16 = outp.tile([P, D_OUT], f16, tag="n16")
            nc.scalar.activation(n16[:], ps[:], Act.Copy,
                                 scale=recip_sb[:, B:B + 1])
            rows = min(P, NC_NODES - B * P)
            nc.sync.dma_start(n1shard_d[B * P:B * P + rows, :], n16[:rows, :])

        def hop1_hook(s):
            if s == 6 and STAGES == "full":
                emit_ag("n_a", n1shard_d[0:HALF, :], ntab_a[:, :])
                emit_copy("n_a", ntabl_a[:, :], ntab_a[:, :])

        if STAGES not in ("mlp", "mlp+ag"):
            emit_hop(htabl_a, htabl_b, ag_insts["h_a"], ag_insts["h_b"],
                     flush1, post_flush_hook=hop1_hook)
        if STAGES == "full":
            emit_ag("n_b", n1shard_d[HALF:NC_NODES, :], ntab_b[:, :])
            emit_copy("n_b", ntabl_b[:, :], ntab_b[:, :])

        def flush2(B, ps):
            mh = outp.tile([P, D_OUT], f32, tag="mh")
            nc.scalar.activation(mh[:], ps[:], Act.Copy,
                                 scale=recip03_sb[:, B:B + 1])
            rows = min(P, NC_NODES - B * P)
            nc.sync.dma_start(mh_out_d[B * P:B * P + rows, :], mh[:rows, :])

        if STAGES == "full":
            emit_hop(ntabl_a, ntabl_b, ag_insts["n_a"], ag_insts["n_b"],
                     flush2, add_prev=True)

    nc.compile()
    return nc


# ---------------------------------------------------------------- entry
def _build_in_maps(x, W, b, metas):
    import ml_dtypes

    eye16 = np.eye(P).astype(np.float16)
    wmat = np.stack([W[0:P, :], W[P:2 * P, :]]).astype(ml_dtypes.bfloat16)
    bias = b.reshape(1, D_OUT).astype(ml_dtypes.bfloat16)
    ones1 = np.ones((1, P), ml_dtypes.bfloat16)

    in_maps = []
    for c in range(NCORES):
        xs = x[c * NC_NODES:(c + 1) * NC_NODES]
        xs_pad = np.zeros((NSB * SBK * P, D_IN), np.float32)
        xs_pad[:NC_NODES] = xs
        xt = np.zeros((NSB, 2, P, SBK * P), ml_dtypes.bfloat16)
        for s in range(NSB):
            chunk = xs_pad[s * SBK * P:(s + 1) * SBK * P]  # [512, 256]
            ct = np.ascontiguousarray(chunk.T)             # [256, 512]
            xt[s, 0] = ct[0:P]
            xt[s, 1] = ct[P:2 * P]
        m = metas[c]
        in_maps.append(
            dict(
                xt=xt, wmat=wmat, bias=bias, ones1=ones1, eye16=eye16,
                oh8=m["oh8"].view(ml_dtypes.float8_e4m3),
                recip=m["recip"], recip03=m["recip03"],
                idx_a=m["idx_a"], idx_b=m["idx_b"],
                wu=np.zeros((P, D_OUT), np.float16),
                wuidx=np.zeros((P, 32), np.int16),
            )
        )
    return in_maps


def kernel(x, W, b, src, dst):
    x = np.asarray(x, np.float32)
    W = np.asarray(W, np.float32)
    b = np.asarray(b, np.float32)
    src = np.asarray(src, np.int32)
    dst = np.asarray(dst, np.int32)

    layout, metas = _build_layout(src, dst)
    key = _layout_key(layout)
    if key not in _PROGRAM_CACHE:
        _PROGRAM_CACHE[key] = _build_program(layout)
    nc = _PROGRAM_CACHE[key]
    in_maps = _build_in_maps(x, W, b, metas)

    from concourse.bass_utils import run_bass_kernel_spmd

    res = run_bass_kernel_spmd(nc, in_maps, list(range(NCORES)))
    h = np.concatenate([res.results[c]["h_out"] for c in range(NCORES)], axis=0)
    mh = np.concatenate([res.results[c]["mh_out"] for c in range(NCORES)], axis=0)
    return (h, mh)
